# revision 31
# baseline (speedup 1.0000x reference)
"""LogGaborConv2d on 8 TRN2 NeuronCores.

Strategy: data-parallel over batch (8 images -> 8 cores). Per core:
- Gabor weights [O=128, I=64, 3, 3] computed on the host (tiny) and fed
  to the device as an fp16 [128, 768] matrix.
- Input is staged as an fp16 column-padded flat stream (width 258) in
  two partition halves: partitions 0:64 hold the stream, partitions
  64:128 hold the same stream shifted by one image row (+258). This
  lets taps (ky=0,kx) and (ky=1,kx) fuse into a single K=128 matmul
  using the full 128x128 PE array. The leftover ky=2 taps run as K=64
  matmuls co-executed pairwise across adjacent windows via
  tile_position row groups.
- Tap-outer ordering over 4-window blocks (4 PSUM banks, double
  buffered) keeps weight loads coherent and the PE queue short.
- fp16 streams everywhere; PSUM accumulates in fp32; output written
  back as fp16 and upconverted on the host.
"""
import math

import numpy as np

import concourse.bacc as bacc
import concourse.bass as bass  # noqa: F401
import concourse.mybir as mybir
import concourse.tile as tile
from concourse.bass_utils import run_bass_kernel_spmd

F32 = mybir.dt.float32
F16 = mybir.dt.float16

# problem constants
NB, C, H, W = 8, 64, 256, 256
O = 128
WP = W + 2            # padded row width
SL = (H + 2) * WP     # padded input stream length (incl. top/bottom pad rows)
OL = H * WP           # padded output stream length: 129 windows of 512
GUARD = 4             # leading guard zeros in the host-side stream
XLEN = 66592          # >= GUARD + SL + slack, mult of 16
TLEN = 512 * 8 + 528  # input tile: 8 windows + halo (517+511 -> 1028 cols max)
TLEN_MINI = 1040      # final window tile
DELTA = 0.001


def build_kernel():
    nc = bacc.Bacc("TRN2", target_bir_lowering=False)
    x = nc.dram_tensor("x", [2 * C, XLEN], F16, kind="ExternalInput")
    w = nc.dram_tensor("w", [2 * C, 768], F16, kind="ExternalInput")
    y = nc.dram_tensor("y", [O, OL], F16, kind="ExternalOutput")

    with tile.TileContext(nc) as tc:
        with (
            tc.tile_pool(name="wg", bufs=1) as wg,
            tc.tile_pool(name="xin", bufs=3) as xin,
            tc.tile_pool(name="outp", bufs=3) as outp,
            tc.tile_pool(name="ps", bufs=2, space="PSUM") as ps,
        ):
            wt = wg.tile([2 * C, 768], F16)
            nc.sync.dma_start(wt[:], w[:])

            # PE warm-up: the tensor engine ramps to full clock only after
            # ~3us of continuous execution. Run throwaway matmuls during
            # the initial DMA wait so the first real matmuls start at speed.
            scr = wg.tile([2 * C, 512], F16)
            nc.gpsimd.memset(scr[:], 0)
            pw = ps.tile([O, 2048], F32, tag="pb", name="pw")
            for _ in range(10):
                nc.tensor.matmul(
                    pw[:, 0:512], scr[:, 0:128], scr[:, 0:512],
                    start=True, stop=True, tile_position=(0, 0),
                )

            def emit_block(pt, xt, q0):
                """4 windows starting at in-tile col q0 (window stride 512).

                pt: one [O, 2048] PSUM tile (4 banks); each matmul writes an
                ISA-legal 512-wide slice. Tap-outer: 3 fused K=128 configs,
                then 3 solo K=64 configs co-executed A/B. One big copy per
                block then drains it, so the PE sees one PSUM-free handshake
                per block instead of four.
                """
                for kx in range(3):
                    lhs = wt[0 : 2 * C, 128 * kx : 128 * kx + 128]
                    for j in range(4):
                        o = q0 + 512 * j + kx - 1
                        nc.tensor.matmul(
                            pt[:, 512 * j : 512 * j + 512], lhs,
                            xt[0 : 2 * C, o : o + 512],
                            start=(kx == 0), stop=False,
                            tile_position=(0, 0),
                        )
                for kx in range(3):
                    lhs_a = wt[0:C, 384 + 128 * kx : 384 + 128 * kx + 128]
                    lhs_b = wt[C : 2 * C, 384 + 128 * kx : 384 + 128 * kx + 128]
                    last = kx == 2
                    for j in range(4):
                        o = q0 + 512 * j + 258 + kx - 1
                        if j % 2 == 0:
                            # A row group reads copy1 at +516 total
                            nc.tensor.matmul(
                                pt[:, 512 * j : 512 * j + 512], lhs_a,
                                xt[0:C, o + 258 : o + 258 + 512],
                                start=False, stop=last,
                                tile_position=(0, 0),
                            )
                        else:
                            # B row group reads copy2 (=stream+258) at +258
                            nc.tensor.matmul(
                                pt[:, 512 * j : 512 * j + 512], lhs_b,
                                xt[C : 2 * C, o : o + 512],
                                start=False, stop=last,
                                tile_position=(64, 0),
                            )

            def emit_mini():
                """final window 128 (unpaired): 3 fused + 3 solo on group A"""
                xtm = xin.tile([2 * C, TLEN], F16, tag="xt", name="xtm")
                nc.sync.dma_start(
                    xtm[:, 0:TLEN_MINI],
                    x[:, 512 * 128 : 512 * 128 + TLEN_MINI],
                )
                pm = ps.tile([O, 2048], F32, tag="pb", name="pm")
                for kx in range(3):
                    nc.tensor.matmul(
                        pm[:, 0:512], wt[0 : 2 * C, 128 * kx : 128 * kx + 128],
                        xtm[0 : 2 * C, GUARD + kx - 1 : GUARD + kx - 1 + 512],
                        start=(kx == 0), stop=False, tile_position=(0, 0),
                    )
                for kx in range(3):
                    o = GUARD + 516 + kx - 1
                    nc.tensor.matmul(
                        pm[:, 0:512],
                        wt[0:C, 384 + 128 * kx : 384 + 128 * kx + 128],
                        xtm[0:C, o : o + 512],
                        start=False, stop=(kx == 2), tile_position=(0, 0),
                    )
                om = outp.tile([O, 4096], F16, tag="ot", name="om")
                nc.scalar.copy(om[:, 0:512], pm[:, 0:512])
                nc.gpsimd.dma_start(y[:, 512 * 128 : 512 * 129], om[:, 0:512])

            CH1, CH2 = 1104, 2640  # first-tile chunk boundaries
            for tblk in range(16):
                w0 = 8 * tblk
                xt = xin.tile([2 * C, TLEN], F16, tag="xt", name="xt")
                if tblk == 0:
                    # split the first load so block 0 can start early
                    nc.sync.dma_start(xt[:, 0:CH1], x[:, 0:CH1])
                    nc.sync.dma_start(xt[:, CH1:CH2], x[:, CH1:CH2])
                    nc.sync.dma_start(xt[:, CH2:TLEN], x[:, CH2:TLEN])
                else:
                    nc.sync.dma_start(xt[:], x[:, 512 * w0 : 512 * w0 + TLEN])
                ot = outp.tile([O, 4096], F16, tag="ot", name="ot")
                for blk in range(2):
                    pt = ps.tile([O, 2048], F32, tag="pb", name="pb")
                    emit_block(pt, xt, GUARD + 2048 * blk)
                    c0 = 2048 * blk
                    if blk == 0:
                        nc.scalar.copy(ot[:, c0 : c0 + 2048], pt[:, 0:2048])
                    else:
                        nc.vector.tensor_copy(
                            ot[:, c0 : c0 + 2048], pt[:, 0:2048]
                        )
                nc.gpsimd.dma_start(y[:, 512 * w0 : 512 * w0 + 4096], ot[:])
                if tblk == 0:
                    emit_mini()

    nc.compile()
    return nc


_NC_CACHE = None


def _get_nc():
    global _NC_CACHE
    if _NC_CACHE is None:
        _NC_CACHE = build_kernel()
    return _NC_CACHE


def _gabor_weights(freq, theta, sigma, psi, f0, theta0, xg, yg):
    """[O, I, 3, 3] float32, matching the reference math."""
    th = theta[:, :, None, None].astype(np.float64)
    sg = sigma[:, :, None, None].astype(np.float64)
    fr = freq[:, :, None, None].astype(np.float64)
    ps = psi[:, :, None, None].astype(np.float64)
    xgd = xg.astype(np.float64)
    ygd = yg.astype(np.float64)
    lf0 = math.log(float(f0[0]))
    th0 = float(theta0[0])
    # rotation preserves radius
    r = np.sqrt(xgd**2 + ygd**2 + DELTA)[None, None]
    g_radial = np.exp(-((np.log(r) - lf0) / (2.0 * (np.log(sg) - lf0))) ** 2)
    g_angular = np.exp(-((th - th0) ** 2) / (2.0 * sg**2))
    g = g_radial * g_angular * np.cos(fr * r + ps) / (2.0 * math.pi * sg**2)
    return g.astype(np.float32)


def kernel(input_tensor, freq, theta, sigma, psi, f0, theta0, xg, yg):
    wfull = _gabor_weights(freq, theta, sigma, psi, f0, theta0, xg, yg)
    wmat = np.zeros((2 * C, 768), np.float16)
    for kx in range(3):
        wmat[0:C, 128 * kx : 128 * kx + 128] = wfull[:, :, 0, kx].T
        wmat[C : 2 * C, 128 * kx : 128 * kx + 128] = wfull[:, :, 1, kx].T
        wmat[0:C, 384 + 128 * kx : 384 + 128 * kx + 128] = wfull[:, :, 2, kx].T
        wmat[C : 2 * C, 384 + 128 * kx : 384 + 128 * kx + 128] = (
            wfull[:, :, 2, kx].T
        )

    x16 = np.asarray(input_tensor, dtype=np.float16)
    nc = _get_nc()
    in_maps = []
    for c in range(NB):
        xp = np.zeros((2 * C, XLEN), np.float16)
        view = xp[0:C, GUARD : GUARD + SL].reshape(C, H + 2, WP)
        view[:, 1 : H + 1, 1 : W + 1] = x16[c]
        xp[C : 2 * C, 0 : XLEN - WP] = xp[0:C, WP:XLEN]
        in_maps.append({"x": xp, "w": wmat})
    res = run_bass_kernel_spmd(nc, in_maps, core_ids=list(range(NB)))
    out = np.empty((NB, O, H, W), np.float32)
    for c in range(NB):
        out[c] = (
            res.results[c]["y"]
            .reshape(O, H, WP)[:, :, 1 : W + 1]
            .astype(np.float32)
        )
    return out


# revision 32
# speedup vs baseline: 1.0142x; 1.0142x over previous
"""LogGaborConv2d on 8 TRN2 NeuronCores.

Strategy: data-parallel over batch (8 images -> 8 cores). Per core:
- Gabor weights [O=128, I=64, 3, 3] computed on the host (tiny) and fed
  to the device as an fp16 [128, 768] matrix.
- Input is staged as an fp16 column-padded flat stream (width 258) in
  two partition halves: partitions 0:64 hold the stream, partitions
  64:128 hold the same stream shifted by one image row (+258). This
  lets taps (ky=0,kx) and (ky=1,kx) fuse into a single K=128 matmul
  using the full 128x128 PE array. The leftover ky=2 taps run as K=64
  matmuls co-executed pairwise across adjacent windows via
  tile_position row groups.
- Tap-outer ordering over 4-window blocks (4 PSUM banks, double
  buffered) keeps weight loads coherent and the PE queue short.
- fp16 streams everywhere; PSUM accumulates in fp32; output written
  back as fp16 and upconverted on the host.
"""
import math

import numpy as np

import concourse.bacc as bacc
import concourse.bass as bass  # noqa: F401
import concourse.mybir as mybir
import concourse.tile as tile
from concourse.bass_utils import run_bass_kernel_spmd

F32 = mybir.dt.float32
F16 = mybir.dt.float16

# problem constants
NB, C, H, W = 8, 64, 256, 256
O = 128
WP = W + 2            # padded row width
SL = (H + 2) * WP     # padded input stream length (incl. top/bottom pad rows)
OL = H * WP           # padded output stream length: 129 windows of 512
GUARD = 4             # leading guard zeros in the host-side stream
XLEN = 66592          # >= GUARD + SL + slack, mult of 16
TLEN = 512 * 8 + 528  # input tile: 8 windows + halo (517+511 -> 1028 cols max)
TLEN_MINI = 1040      # final window tile
DELTA = 0.001


def build_kernel():
    nc = bacc.Bacc("TRN2", target_bir_lowering=False)
    x = nc.dram_tensor("x", [2 * C, XLEN], F16, kind="ExternalInput")
    w = nc.dram_tensor("w", [2 * C, 768], F16, kind="ExternalInput")
    y = nc.dram_tensor("y", [O, OL], F16, kind="ExternalOutput")

    with tile.TileContext(nc) as tc:
        with (
            tc.tile_pool(name="wg", bufs=1) as wg,
            tc.tile_pool(name="xin", bufs=3) as xin,
            tc.tile_pool(name="outp", bufs=3) as outp,
            tc.tile_pool(name="ps", bufs=2, space="PSUM") as ps,
        ):
            wt = wg.tile([2 * C, 768], F16)
            nc.sync.dma_start(wt[:], w[:])

            # PE warm-up: the tensor engine ramps to full clock only after
            # ~3us of continuous execution. Run throwaway matmuls during
            # the initial DMA wait so the first real matmuls start at speed.
            scr = wg.tile([2 * C, 512], F16)
            nc.gpsimd.memset(scr[:], 0)
            pw = ps.tile([O, 512], F32, tag="p0", name="pw")
            for _ in range(8):
                nc.tensor.matmul(
                    pw[:], scr[:, 0:128], scr[:, 0:512],
                    start=True, stop=True, tile_position=(0, 0),
                )

            def copy_engine(i):
                return (nc.scalar.copy, nc.vector.tensor_copy)[i % 2]

            def emit_block(pt, xt, q0):
                """4 windows starting at in-tile col q0 (window stride 512).

                pt: list of 4 PSUM tiles. Tap-outer: 3 fused K=128 configs,
                then 3 solo K=64 configs co-executed A/B.
                """
                for kx in range(3):
                    lhs = wt[0 : 2 * C, 128 * kx : 128 * kx + 128]
                    for j in range(4):
                        o = q0 + 512 * j + kx - 1
                        nc.tensor.matmul(
                            pt[j][:], lhs, xt[0 : 2 * C, o : o + 512],
                            start=(kx == 0), stop=False,
                            tile_position=(0, 0),
                        )
                for kx in range(3):
                    lhs_a = wt[0:C, 384 + 128 * kx : 384 + 128 * kx + 128]
                    lhs_b = wt[C : 2 * C, 384 + 128 * kx : 384 + 128 * kx + 128]
                    last = kx == 2
                    for j in range(4):
                        o = q0 + 512 * j + 258 + kx - 1
                        if j % 2 == 0:
                            # A row group reads copy1 at +516 total
                            nc.tensor.matmul(
                                pt[j][:], lhs_a,
                                xt[0:C, o + 258 : o + 258 + 512],
                                start=False, stop=last,
                                tile_position=(0, 0),
                            )
                        else:
                            # B row group reads copy2 (=stream+258) at +258
                            nc.tensor.matmul(
                                pt[j][:], lhs_b,
                                xt[C : 2 * C, o : o + 512],
                                start=False, stop=last,
                                tile_position=(64, 0),
                            )

            def emit_mini():
                """final window 128 (unpaired): 3 fused + 3 solo on group A"""
                xtm = xin.tile([2 * C, TLEN], F16, tag="xt", name="xtm")
                nc.sync.dma_start(
                    xtm[:, 0:TLEN_MINI],
                    x[:, 512 * 128 : 512 * 128 + TLEN_MINI],
                )
                pm = ps.tile([O, 512], F32, tag="p0", name="pm")
                for kx in range(3):
                    nc.tensor.matmul(
                        pm[:], wt[0 : 2 * C, 128 * kx : 128 * kx + 128],
                        xtm[0 : 2 * C, GUARD + kx - 1 : GUARD + kx - 1 + 512],
                        start=(kx == 0), stop=False, tile_position=(0, 0),
                    )
                for kx in range(3):
                    o = GUARD + 516 + kx - 1
                    nc.tensor.matmul(
                        pm[:], wt[0:C, 384 + 128 * kx : 384 + 128 * kx + 128],
                        xtm[0:C, o : o + 512],
                        start=False, stop=(kx == 2), tile_position=(0, 0),
                    )
                om = outp.tile([O, 4096], F16, tag="ot", name="om")
                nc.scalar.copy(om[:, 0:512], pm[:])
                nc.gpsimd.dma_start(y[:, 512 * 128 : 512 * 129], om[:, 0:512])

            CH1, CH2 = 1104, 2640  # first-tile chunk boundaries
            for tblk in range(16):
                w0 = 8 * tblk
                xt = xin.tile([2 * C, TLEN], F16, tag="xt", name="xt")
                if tblk == 0:
                    # split the first load so block 0 can start early
                    nc.sync.dma_start(xt[:, 0:CH1], x[:, 0:CH1])
                    nc.sync.dma_start(xt[:, CH1:CH2], x[:, CH1:CH2])
                    nc.sync.dma_start(xt[:, CH2:TLEN], x[:, CH2:TLEN])
                else:
                    nc.sync.dma_start(xt[:], x[:, 512 * w0 : 512 * w0 + TLEN])
                ot = outp.tile([O, 4096], F16, tag="ot", name="ot")
                for blk in range(2):
                    pt = [
                        ps.tile([O, 512], F32, tag=f"p{j}", name=f"p{j}")
                        for j in range(4)
                    ]
                    emit_block(pt, xt, GUARD + 2048 * blk)
                    for j in range(4):
                        c0 = 2048 * blk + 512 * j
                        copy_engine(4 * blk + j)(
                            ot[:, c0 : c0 + 512], pt[j][:]
                        )
                nc.gpsimd.dma_start(y[:, 512 * w0 : 512 * w0 + 4096], ot[:])
                if tblk == 0:
                    emit_mini()

    nc.compile()
    return nc


_NC_CACHE = None


def _get_nc():
    global _NC_CACHE
    if _NC_CACHE is None:
        _NC_CACHE = build_kernel()
    return _NC_CACHE


def _gabor_weights(freq, theta, sigma, psi, f0, theta0, xg, yg):
    """[O, I, 3, 3] float32, matching the reference math."""
    th = theta[:, :, None, None].astype(np.float64)
    sg = sigma[:, :, None, None].astype(np.float64)
    fr = freq[:, :, None, None].astype(np.float64)
    ps = psi[:, :, None, None].astype(np.float64)
    xgd = xg.astype(np.float64)
    ygd = yg.astype(np.float64)
    lf0 = math.log(float(f0[0]))
    th0 = float(theta0[0])
    # rotation preserves radius
    r = np.sqrt(xgd**2 + ygd**2 + DELTA)[None, None]
    g_radial = np.exp(-((np.log(r) - lf0) / (2.0 * (np.log(sg) - lf0))) ** 2)
    g_angular = np.exp(-((th - th0) ** 2) / (2.0 * sg**2))
    g = g_radial * g_angular * np.cos(fr * r + ps) / (2.0 * math.pi * sg**2)
    return g.astype(np.float32)


def kernel(input_tensor, freq, theta, sigma, psi, f0, theta0, xg, yg):
    wfull = _gabor_weights(freq, theta, sigma, psi, f0, theta0, xg, yg)
    wmat = np.zeros((2 * C, 768), np.float16)
    for kx in range(3):
        wmat[0:C, 128 * kx : 128 * kx + 128] = wfull[:, :, 0, kx].T
        wmat[C : 2 * C, 128 * kx : 128 * kx + 128] = wfull[:, :, 1, kx].T
        wmat[0:C, 384 + 128 * kx : 384 + 128 * kx + 128] = wfull[:, :, 2, kx].T
        wmat[C : 2 * C, 384 + 128 * kx : 384 + 128 * kx + 128] = (
            wfull[:, :, 2, kx].T
        )

    x16 = np.asarray(input_tensor, dtype=np.float16)
    nc = _get_nc()
    in_maps = []
    for c in range(NB):
        xp = np.zeros((2 * C, XLEN), np.float16)
        view = xp[0:C, GUARD : GUARD + SL].reshape(C, H + 2, WP)
        view[:, 1 : H + 1, 1 : W + 1] = x16[c]
        xp[C : 2 * C, 0 : XLEN - WP] = xp[0:C, WP:XLEN]
        in_maps.append({"x": xp, "w": wmat})
    res = run_bass_kernel_spmd(nc, in_maps, core_ids=list(range(NB)))
    out = np.empty((NB, O, H, W), np.float32)
    for c in range(NB):
        out[c] = (
            res.results[c]["y"]
            .reshape(O, H, WP)[:, :, 1 : W + 1]
            .astype(np.float32)
        )
    return out
